# revision 10
# baseline (speedup 1.0000x reference)
"""NeuralCache lookup kernel for 8 Trainium2 NeuronCores.

Strategy: data-parallel over the batch (512 rows/core), hash tables
replicated on every core.  Host packs reliability/valid/last_access into an
augmented key-row table so a single indirect row-gather per (row, table)
fetches the key vector and all per-address metadata.  On device:
  PE      : LSH projection matmuls + bit-pack matmul-free reduce
  DVE     : fused (x * rsqrt) * key + sum  (scalar_tensor_tensor accum)
  ACT     : sum-of-squares (Square + accum_out), sqrt
  GPSIMD  : indirect DMA gathers (keys+meta rows, value rows)
Outputs: values (bf16, exact), and a packed [16,128] f32 meta tensor
(hit/max_sim/hit_addr/hit_table per row) which the host unpacks/casts.
"""

import sys

if "/opt/trn_rl_repo" not in sys.path:
    sys.path.insert(0, "/opt/trn_rl_repo")

from contextlib import ExitStack
from dataclasses import dataclass

import ml_dtypes
import numpy as np

import concourse.bass as bass
import concourse.mybir as mybir
import concourse.tile as tile
from concourse import bacc
from concourse.bass import IndirectOffsetOnAxis

F32 = mybir.dt.float32
BF16 = mybir.dt.bfloat16
I32 = mybir.dt.int32
ALU = mybir.AluOpType
ACTF = mybir.ActivationFunctionType

NEG_BIG = -1.0e30
IDX_BIG = 1024.0  # must keep iota + IDX_BIG exact in f32


@dataclass(frozen=True)
class Cfg:
    hash_bits: int = 16
    T: int = 4
    D: int = 1024
    O: int = 1024
    ram: int = 65536
    bcore: int = 512  # batch rows per core
    now: float = 1.0
    key_sim_threshold: float = 0.0

    @property
    def rt(self):  # row tiles per core
        return self.bcore // 128

    @property
    def rows(self):  # total table rows
        return self.T * self.ram

    @property
    def tb(self):  # total hash bits across tables
        return self.T * self.hash_bits

    @property
    def augw(self):  # augmented row width (bf16 elems), 16B aligned
        w = self.O + 8
        return (w + 7) // 8 * 8

    @property
    def dj(self):  # number of 128-partition tiles along D
        return self.D // 128


def _sq(ap):
    """Canonicalize an AP with singleton dims to 2D [P, free]."""
    shp = ap.shape
    if len(shp) == 2:
        return ap
    names = " ".join(f"a{i}" for i in range(1, len(shp)))
    return ap.rearrange(f"p {names} -> p ({names})")


def build_consts(cfg: Cfg) -> np.ndarray:
    """Constant tensor [128, C] f32 shared by all cores.

    cols 0:TB                 bit weights 2^(c % hash_bits)
    next TB4=4*rt cols        table offset  (c%T) * ram
    next TB4 cols             iota          t = c%T
    next TB4 cols             iota + IDX_BIG
    next TB4 cols             NEG_BIG
    last 128 cols             identity
    """
    TB = cfg.tb
    n4 = cfg.rt * cfg.T
    C = TB + 4 * n4 + 128
    c = np.zeros((128, C), np.float32)
    col = 0
    c[:, col : col + TB] = (2.0 ** (np.arange(TB) % cfg.hash_bits))[None, :]
    col += TB
    t_of = np.arange(n4) % cfg.T
    c[:, col : col + n4] = (t_of * cfg.ram)[None, :]
    col += n4
    c[:, col : col + n4] = t_of[None, :]
    col += n4
    c[:, col : col + n4] = (t_of + IDX_BIG)[None, :]
    col += n4
    c[:, col : col + n4] = NEG_BIG
    col += n4
    c[:, col : col + 128] = np.eye(128, dtype=np.float32)
    return c


def const_slices(cfg: Cfg):
    TB = cfg.tb
    n4 = cfg.rt * cfg.T
    o = {}
    o["W"] = (0, TB)
    o["TBLOFF"] = (TB, TB + n4)
    o["IOTA"] = (TB + n4, TB + 2 * n4)
    o["IOTABIG"] = (TB + 2 * n4, TB + 3 * n4)
    o["NEGBIG"] = (TB + 3 * n4, TB + 4 * n4)
    o["ID"] = (TB + 4 * n4, TB + 4 * n4 + 128)
    return o


def build_program(cfg: Cfg, enable_asserts: bool = False, debug_taps: bool = False):
    """Build and compile the per-core Bass program (identical on all cores)."""
    nc = bacc.Bacc(
        "TRN2",
        target_bir_lowering=False,
        debug=False,
        enable_asserts=enable_asserts,
    )
    T, D, O, RT = cfg.T, cfg.D, cfg.O, cfg.rt
    HB, TB, AW, DJ = cfg.hash_bits, cfg.tb, cfg.augw, cfg.dj
    N4 = RT * T
    CS = const_slices(cfg)
    CW = CS["ID"][1]

    x_d = nc.dram_tensor("x", [cfg.bcore, D], F32, kind="ExternalInput")
    xT_d = nc.dram_tensor("xT", [D, cfg.bcore], F32, kind="ExternalInput")
    pl_d = nc.dram_tensor("planes_r", [D, TB], F32, kind="ExternalInput")
    aug_d = nc.dram_tensor("aug", [cfg.rows, AW], BF16, kind="ExternalInput")
    val_d = nc.dram_tensor("vals", [cfg.rows + 1, O], BF16, kind="ExternalInput")
    cst_d = nc.dram_tensor("consts", [128, CW], F32, kind="ExternalInput")

    vo_d = nc.dram_tensor("values_out", [cfg.bcore, O], BF16, kind="ExternalOutput")
    mo_d = nc.dram_tensor("meta_out", [4 * RT, 128], F32, kind="ExternalOutput")

    def cs(tile_, name):
        a, b = CS[name]
        return tile_[:, a:b]

    with tile.TileContext(nc) as tc, ExitStack() as ctx:
        inp = ctx.enter_context(tc.tile_pool(name="inp", bufs=1))
        small = ctx.enter_context(tc.tile_pool(name="small", bufs=1))
        kgp = ctx.enter_context(tc.tile_pool(name="kg", bufs=RT))
        vgp = ctx.enter_context(tc.tile_pool(name="vg", bufs=RT))
        scr = ctx.enter_context(tc.tile_pool(name="scr", bufs=2))
        psum = ctx.enter_context(tc.tile_pool(name="ps", bufs=2, space="PSUM"))
        psum2 = ctx.enter_context(tc.tile_pool(name="ps2", bufs=1, space="PSUM"))

        # ---- load inputs ----
        cst = inp.tile([128, CW], F32)
        nc.sync.dma_start(cst[:], cst_d.ap())
        pl = inp.tile([128, DJ, TB], F32)
        nc.sync.dma_start(pl[:], pl_d.ap().rearrange("(j p) h -> p j h", p=128))
        xT = inp.tile([128, DJ, cfg.bcore], F32)
        nc.sync.dma_start(xT[:], xT_d.ap().rearrange("(j p) b -> p j b", p=128))
        xt = inp.tile([128, RT, D], F32)
        nc.sync.dma_start(xt[:], x_d.ap().rearrange("(r p) d -> p r d", p=128))

        # persistent small tiles, column layout [128, N4] with col = T*r + t
        addr16 = small.tile([128, N4], F32, tag="addr16")
        gidxf = small.tile([128, N4], F32, tag="gidxf")
        gidxi = small.tile([128, N4], I32, tag="gidxi")
        sims = small.tile([128, N4], F32, tag="sims")
        rel = small.tile([128, N4], F32, tag="rel")
        vld = small.tile([128, N4], F32, tag="vld")
        la = small.tile([128, N4], F32, tag="la")
        ssq = small.tile([128, RT], F32, tag="ssq")
        rs = small.tile([128, RT], F32, tag="rs")

        # ---- hash: proj, sign bits, packed addresses ----
        for r in range(RT):
            proj_ps = psum.tile([128, TB], F32, tag="proj")
            for j in range(DJ):
                nc.tensor.matmul(
                    out=proj_ps[:],
                    lhsT=xT[:, j, 128 * r : 128 * (r + 1)],
                    rhs=pl[:, j, :],
                    start=(j == 0),
                    stop=(j == DJ - 1),
                )
            bits = scr.tile([128, TB], F32, tag="bits")
            nc.vector.tensor_scalar(
                out=bits[:], in0=proj_ps[:], scalar1=0.0, scalar2=None, op0=ALU.is_gt
            )
            bw = scr.tile([128, TB], F32, tag="bw")
            nc.vector.tensor_mul(bw[:], bits[:], cs(cst, "W"))
            # reduce each table's hash_bits chunk -> addresses
            nc.vector.tensor_reduce(
                out=addr16[:, T * r : T * (r + 1)],
                in_=bw[:].rearrange("p (t h) -> p t h", h=HB),
                axis=mybir.AxisListType.X,
                op=ALU.add,
            )

        # global row index = addr + t*ram
        nc.vector.tensor_add(gidxf[:], addr16[:], cs(cst, "TBLOFF"))
        nc.vector.tensor_copy(gidxi[:], gidxf[:])

        # ---- sum of squares + rsqrt (ACT + DVE + one Newton step) ----
        for r in range(RT):
            sq = scr.tile([128, D], F32, tag="sq")
            nc.scalar.activation(
                out=sq[:],
                in_=xt[:, r, :],
                func=ACTF.Square,
                accum_out=ssq[:, r : r + 1],
            )
        ssqc = small.tile([128, RT], F32, tag="ssqc")
        nc.vector.tensor_scalar_max(ssqc[:], ssq[:], 1e-24)
        sqv = small.tile([128, RT], F32, tag="sqv")
        nc.scalar.activation(out=sqv[:], in_=ssqc[:], func=ACTF.Sqrt)
        y0 = small.tile([128, RT], F32, tag="y0")
        nc.vector.reciprocal(y0[:], sqv[:])
        # one Newton iteration: y = y0 * (1.5 - 0.5 * ssqc * y0^2)
        n1 = small.tile([128, RT], F32, tag="n1")
        nc.vector.tensor_mul(n1[:], ssqc[:], y0[:])
        nc.vector.tensor_mul(n1[:], n1[:], y0[:])
        nc.vector.tensor_scalar(
            out=n1[:], in0=n1[:], scalar1=-0.5, scalar2=1.5, op0=ALU.mult, op1=ALU.add
        )
        nc.vector.tensor_mul(rs[:], y0[:], n1[:])

        # ---- gather augmented key rows; fused normalized dot products ----
        for r in range(RT):
            # one [128, T*AW] tile; 4 single-offset indirect gathers into
            # column slices (multi-offset indirect DMA is broken on HW)
            kg = kgp.tile([128, T * AW], BF16, tag="kg")
            for t in range(T):
                nc.gpsimd.indirect_dma_start(
                    out=kg[:, t * AW : (t + 1) * AW],
                    out_offset=None,
                    in_=aug_d.ap(),
                    in_offset=IndirectOffsetOnAxis(
                        ap=gidxi[:, T * r + t : T * r + t + 1], axis=0
                    ),
                )
            kg3 = kg[:].rearrange("p (t w) -> p t w", w=AW)
            for t in range(T):
                dot = scr.tile([128, D], F32, tag="dot")
                nc.vector.scalar_tensor_tensor(
                    out=dot[:],
                    in0=xt[:, r, :],
                    scalar=rs[:, r : r + 1],
                    in1=_sq(kg3[:, t, 0:D]),
                    op0=ALU.mult,
                    op1=ALU.mult,
                    accum_out=sims[:, T * r + t : T * r + t + 1],
                )
            # metadata: rel = hi+lo, valid, la = hi+lo   (strided [128, T] APs)
            nc.vector.tensor_add(
                rel[:, T * r : T * (r + 1)],
                _sq(kg3[:, :, O + 0 : O + 1]),
                _sq(kg3[:, :, O + 1 : O + 2]),
            )
            nc.vector.tensor_copy(
                vld[:, T * r : T * (r + 1)], _sq(kg3[:, :, O + 2 : O + 3])
            )
            nc.vector.tensor_add(
                la[:, T * r : T * (r + 1)],
                _sq(kg3[:, :, O + 3 : O + 4]),
                _sq(kg3[:, :, O + 4 : O + 5]),
            )

        # ---- per-row selection logic, batched over all row tiles ----
        def g3(ap):  # view [128, N4] as [128, RT, T]
            return ap.rearrange("p (r t) -> p r t", t=T)

        def bc4(ap):  # broadcast [128, RT] -> [128, RT, T]
            return ap.to_broadcast([128, RT, T])

        lg = small
        ge0 = lg.tile([128, N4], F32, tag="ge0")
        nc.vector.tensor_scalar(
            out=ge0[:], in0=sims[:], scalar1=cfg.key_sim_threshold, scalar2=None,
            op0=ALU.is_ge,
        )
        hit = lg.tile([128, N4], F32, tag="hit")
        nc.vector.tensor_mul(hit[:], ge0[:], vld[:])
        hitm = lg.tile([128, RT], F32, tag="hitm")
        nc.vector.tensor_reduce(
            out=hitm[:], in_=g3(hit[:]), axis=mybir.AxisListType.X, op=ALU.max
        )
        # sm = sims where hit else -1e30, built exactly:
        #   d = hit*1e30 - 1e30  (0 or -1e30, exact)   sm = d + sims*hit
        sm = lg.tile([128, N4], F32, tag="sm")
        smd = lg.tile([128, N4], F32, tag="smd")
        nc.vector.tensor_scalar(
            out=smd[:], in0=hit[:], scalar1=-NEG_BIG, scalar2=NEG_BIG,
            op0=ALU.mult, op1=ALU.add,
        )
        nc.vector.tensor_mul(sm[:], sims[:], hit[:])
        nc.vector.tensor_add(sm[:], sm[:], smd[:])
        mx = lg.tile([128, RT], F32, tag="mx")
        nc.vector.tensor_reduce(
            out=mx[:], in_=g3(sm[:]), axis=mybir.AxisListType.X, op=ALU.max
        )
        eq = lg.tile([128, N4], F32, tag="eq")
        nc.vector.tensor_tensor(eq[:], g3(sm[:]), bc4(mx[:]), op=ALU.is_equal)
        tmp = lg.tile([128, N4], F32, tag="tmp")
        nc.vector.scalar_tensor_tensor(
            out=tmp[:], in0=eq[:], scalar=-IDX_BIG, in1=cs(cst, "IOTABIG"),
            op0=ALU.mult, op1=ALU.add,
        )
        opb = lg.tile([128, RT], F32, tag="opb")
        nc.vector.tensor_reduce(
            out=opb[:], in_=g3(tmp[:]), axis=mybir.AxisListType.X, op=ALU.min
        )
        oh = lg.tile([128, N4], F32, tag="oh")
        nc.vector.tensor_tensor(
            oh[:], g3(cs(cst, "IOTA")), bc4(opb[:]), op=ALU.is_equal
        )
        og = lg.tile([128, N4], F32, tag="og")
        nc.vector.tensor_mul(og[:], oh[:], gidxf[:])
        opg = lg.tile([128, RT], F32, tag="opg")
        nc.vector.tensor_reduce(
            out=opg[:], in_=g3(og[:]), axis=mybir.AxisListType.X, op=ALU.add
        )
        # miss rows -> zero row (index cfg.rows)
        vidxf = lg.tile([128, RT], F32, tag="vidxf")
        nc.vector.tensor_scalar_add(vidxf[:], opg[:], -float(cfg.rows))
        nc.vector.tensor_mul(vidxf[:], vidxf[:], hitm[:])
        nc.vector.tensor_scalar_add(vidxf[:], vidxf[:], float(cfg.rows))
        vidxi = lg.tile([128, RT], I32, tag="vidxi")
        nc.vector.tensor_copy(vidxi[:], vidxf[:])

        # recency + blended score
        rec = lg.tile([128, N4], F32, tag="rec")
        nc.vector.tensor_scalar(
            out=rec[:], in0=la[:], scalar1=-1.0, scalar2=1.0,
            op0=ALU.mult, op1=ALU.add,
        )
        nc.vector.tensor_scalar_max(rec[:], rec[:], 0.0)
        nc.vector.tensor_scalar_add(rec[:], rec[:], 1.0)
        nc.vector.reciprocal(rec[:], rec[:])
        bl = lg.tile([128, N4], F32, tag="bl")
        nc.vector.tensor_scalar_mul(bl[:], rec[:], 0.1)
        nc.vector.scalar_tensor_tensor(
            out=bl[:], in0=rel[:], scalar=0.2, in1=bl[:], op0=ALU.mult, op1=ALU.add
        )
        nc.vector.scalar_tensor_tensor(
            out=bl[:], in0=sims[:], scalar=0.7, in1=bl[:], op0=ALU.mult, op1=ALU.add
        )
        mxb = lg.tile([128, RT], F32, tag="mxb")
        nc.vector.tensor_reduce(
            out=mxb[:], in_=g3(bl[:]), axis=mybir.AxisListType.X, op=ALU.max
        )
        eqb = lg.tile([128, N4], F32, tag="eqb")
        nc.vector.tensor_tensor(eqb[:], g3(bl[:]), bc4(mxb[:]), op=ALU.is_equal)
        nc.vector.scalar_tensor_tensor(
            out=eqb[:], in0=eqb[:], scalar=-IDX_BIG, in1=cs(cst, "IOTABIG"),
            op0=ALU.mult, op1=ALU.add,
        )
        btb = lg.tile([128, RT], F32, tag="btb")
        nc.vector.tensor_reduce(
            out=btb[:], in_=g3(eqb[:]), axis=mybir.AxisListType.X, op=ALU.min
        )
        ohb = lg.tile([128, N4], F32, tag="ohb")
        nc.vector.tensor_tensor(
            ohb[:], g3(cs(cst, "IOTA")), bc4(btb[:]), op=ALU.is_equal
        )
        bsc = lg.tile([128, N4], F32, tag="bsc")
        nc.vector.tensor_mul(bsc[:], ohb[:], sims[:])
        bs = lg.tile([128, RT], F32, tag="bs")
        nc.vector.tensor_reduce(
            out=bs[:], in_=g3(bsc[:]), axis=mybir.AxisListType.X, op=ALU.add
        )
        nc.vector.tensor_mul(bsc[:], ohb[:], addr16[:])
        ba = lg.tile([128, RT], F32, tag="ba")
        nc.vector.tensor_reduce(
            out=ba[:], in_=g3(bsc[:]), axis=mybir.AxisListType.X, op=ALU.add
        )

        # pack meta outputs [128, 16]: cols 4f + r  (f: hit, max_sim, addr, table)
        mp = lg.tile([128, 4 * RT], F32, tag="mp")
        nc.vector.tensor_copy(mp[:, 0:RT], hitm[:])
        nc.vector.tensor_mul(mp[:, RT : 2 * RT], bs[:], hitm[:])
        # hit_addrs = addr0 + (best_addr - addr0) * hit
        addr0 = g3(addr16[:])[:, :, 0:1].rearrange("p r x -> p (r x)")
        had = lg.tile([128, RT], F32, tag="had")
        nc.vector.tensor_sub(had[:], ba[:], addr0)
        nc.vector.tensor_mul(had[:], had[:], hitm[:])
        nc.vector.tensor_add(mp[:, 2 * RT : 3 * RT], had[:], addr0)
        nc.vector.tensor_mul(mp[:, 3 * RT : 4 * RT], btb[:], hitm[:])

        # transpose meta -> [16, 128] and store
        mps = psum2.tile([4 * RT, 128], F32, tag="mt")
        nc.tensor.transpose(out=mps[:], in_=mp[:], identity=cs(cst, "ID"))
        mts = lg.tile([4 * RT, 128], F32, tag="mts")
        nc.scalar.copy(mts[:], mps[:])
        nc.sync.dma_start(mo_d.ap()[0 : 4 * RT, :], mts[:])

        if debug_taps:
            taps = {
                "addr16": addr16, "gidxf": gidxf, "sims": sims, "rel": rel,
                "vld": vld, "la": la, "hit": hit, "sm": sm, "mx": mx,
                "eq": eq, "opb": opb, "oh": oh, "opg": opg, "vidxf": vidxf,
                "rec": rec, "bl": bl, "btb": btb, "bs": bs, "ba": ba,
                "rs": rs, "ssq": ssq, "hitm": hitm,
            }
            for nm, tl in taps.items():
                w = tl.shape[1]
                d = nc.dram_tensor(f"dbg_{nm}", [128, w], F32, kind="ExternalOutput")
                dtap = lg.tile([128, w], F32, tag=f"dt_{nm}")
                nc.vector.tensor_copy(dtap[:], tl[:])
                nc.sync.dma_start(d.ap(), dtap[:])

        # ---- gather winning value rows, store ----
        vo_r = vo_d.ap().rearrange("(r p) d -> p r d", p=128)
        for r in range(RT):
            vg = vgp.tile([128, O], BF16, tag="vgt")
            nc.gpsimd.indirect_dma_start(
                out=vg[:],
                out_offset=None,
                in_=val_d.ap(),
                in_offset=IndirectOffsetOnAxis(ap=vidxi[:, r : r + 1], axis=0),
            )
            nc.sync.dma_start(vo_r[:, r, :], vg[:])

    nc.compile()
    return nc


def host_prep(cfg: Cfg, x, planes, keys, values, valid, reliability, last_access):
    """Build per-core input maps from full inputs."""
    f32 = np.float32
    bf16 = ml_dtypes.bfloat16
    x = np.ascontiguousarray(np.asarray(x, dtype=f32))
    planes = np.asarray(planes, dtype=f32)
    keys = np.asarray(keys).astype(bf16, copy=False)
    values = np.asarray(values).astype(bf16, copy=False)
    valid = np.asarray(valid)
    rel = np.asarray(reliability, dtype=f32).reshape(-1)
    la = np.asarray(last_access, dtype=f32).reshape(-1)

    R, O, AW = cfg.rows, cfg.O, cfg.augw
    aug = np.zeros((R, AW), bf16)
    aug[:, : cfg.O] = keys.reshape(R, cfg.D)
    rel_hi = rel.astype(bf16)
    rel_lo = (rel - rel_hi.astype(f32)).astype(bf16)
    la_hi = la.astype(bf16)
    la_lo = (la - la_hi.astype(f32)).astype(bf16)
    aug[:, O + 0] = rel_hi
    aug[:, O + 1] = rel_lo
    aug[:, O + 2] = valid.reshape(-1).astype(bf16)
    aug[:, O + 3] = la_hi
    aug[:, O + 4] = la_lo

    vals2 = np.zeros((R + 1, O), bf16)
    vals2[:R] = values.reshape(R, O)

    planes_r = np.ascontiguousarray(
        planes.transpose(1, 0, 2).reshape(cfg.D, cfg.tb)
    )
    consts = build_consts(cfg)

    n_cores = x.shape[0] // cfg.bcore
    in_maps = []
    for c in range(n_cores):
        xs = np.ascontiguousarray(x[c * cfg.bcore : (c + 1) * cfg.bcore])
        in_maps.append(
            {
                "x": xs,
                "xT": np.ascontiguousarray(xs.T),
                "planes_r": planes_r,
                "aug": aug,
                "vals": vals2,
                "consts": consts,
            }
        )
    return in_maps


def assemble_outputs(cfg: Cfg, results):
    """Stitch per-core results into full outputs (matching reference dtypes)."""
    vo = np.concatenate(
        [np.asarray(r["values_out"], dtype=np.float32) for r in results], axis=0
    )
    hits, msims, haddrs, htbls = [], [], [], []
    for r in results:
        m = np.asarray(r["meta_out"], dtype=np.float32).reshape(4, cfg.rt, 128)
        hits.append(m[0].reshape(-1))
        msims.append(m[1].reshape(-1))
        haddrs.append(m[2].reshape(-1))
        htbls.append(m[3].reshape(-1))
    hit_mask = np.concatenate(hits) > 0.5
    max_sim = np.concatenate(msims).astype(np.float32)
    hit_addrs = np.rint(np.concatenate(haddrs)).astype(np.int32)
    hit_tables = np.rint(np.concatenate(htbls)).astype(np.int32)
    return vo, hit_mask, max_sim, hit_addrs, hit_tables


_PROGRAM_CACHE: dict = {}


def get_program(cfg: Cfg, enable_asserts: bool = False):
    key = (cfg, enable_asserts)
    if key not in _PROGRAM_CACHE:
        _PROGRAM_CACHE[key] = build_program(cfg, enable_asserts)
    return _PROGRAM_CACHE[key]


def kernel(
    x, planes, keys, values, valid, reliability, last_access,
    _trace=False, _tmpdir=None,
):
    from concourse.bass_utils import run_bass_kernel_spmd

    cfg = Cfg()
    n_cores = 8
    assert np.asarray(x).shape == (cfg.bcore * n_cores, cfg.D)
    nc = get_program(cfg)
    in_maps = host_prep(
        cfg, x, planes, keys, values, valid, reliability, last_access
    )
    res = run_bass_kernel_spmd(
        nc, in_maps, core_ids=list(range(n_cores)), trace=_trace, tmpdir=_tmpdir
    )
    out = assemble_outputs(cfg, res.results)
    if _trace:
        kernel.last_results = res
    return out
